# revision 2
# baseline (speedup 1.0000x reference)
"""Multi-head attention (B=4, S=2048, E=1024, H=16, D=64) on 8 TRN2 NeuronCores.

Sharding: core c handles batch b = c//2 and heads [8*(c%2), 8*(c%2)+8) —
data parallel over batch, tensor parallel over heads. No collectives:
each core computes its own output slice, gathered on host.

Per-core algorithm (all matmuls in float32r = full-rate fp32):
  qT = (Wq_slice)^T-free matmul:  qT[f, s]  = sum_e Wq[e, f] * XqT[e, s]
  kT likewise; v[s, f] = sum_e XvT[e, s] * Wv[e, f]  (natural layout)
  per head h, sq-chunk j (512 wide):
    S^T[sk_blk, sq] = matmul(lhsT=kT_h[:, blk], rhs=qT_h[:, j])   (K=64)
    P = exp(S^T / 8)            (ACT, batched over 2 psum banks)
    ctx^T[0:64, sq] += matmul(lhsT=[v_h | 1], rhs=P)  -> row 64 = sum(P)
  output per core: [8 heads, 65, 2048]; host divides rows 0..63 by row 64,
  transposes, and scatters into the full [4, 2048, 1024] result.
"""

import numpy as np
from contextlib import ExitStack

import concourse.bass as bass
import concourse.tile as tile
from concourse import bacc
from concourse import mybir
from concourse.bass_utils import run_bass_kernel_spmd

F32 = mybir.dt.float32
F32R = mybir.dt.float32r
EXP = mybir.ActivationFunctionType.Exp

B, S, E = 4, 2048, 1024
H, D = 16, 64
HPC = 8            # heads per core
FPC = HPC * D      # 512 output features per core
N_CORES = 8
KC = E // 128      # contraction chunks
NJ = S // 512      # sq chunks
NT = S // 128      # sk blocks
SCALE = 0.125      # 1/sqrt(64)


def build_bass():
    nc = bacc.Bacc()
    xq = nc.declare_dram_parameter("xq_t", [E, S], F32R, isOutput=False)
    xk = nc.declare_dram_parameter("xk_t", [E, S], F32R, isOutput=False)
    xv = nc.declare_dram_parameter("xv_t", [E, S], F32R, isOutput=False)
    wq = nc.declare_dram_parameter("wq", [E, FPC], F32R, isOutput=False)
    wk = nc.declare_dram_parameter("wk", [E, FPC], F32R, isOutput=False)
    wv = nc.declare_dram_parameter("wv", [E, FPC], F32R, isOutput=False)
    out = nc.declare_dram_parameter("out", [HPC, D + 1, S], F32, isOutput=True)

    with tile.TileContext(nc) as tc, ExitStack() as ctx:
        sb = ctx.enter_context(tc.tile_pool(name="sb", bufs=1))
        xs = ctx.enter_context(tc.tile_pool(name="xs", bufs=2))
        exp = ctx.enter_context(tc.tile_pool(name="exp", bufs=2))
        ps = ctx.enter_context(tc.tile_pool(name="ps", bufs=2, space="PSUM"))

        # --- weights, resident ---
        w_sb = {}
        for name, w in (("wq", wq), ("wk", wk), ("wv", wv)):
            t = sb.tile([128, KC, FPC], F32R, name=f"{name}_sb", tag=f"{name}_sb")
            nc.sync.dma_start(out=t, in_=w.rearrange("(kc p) f -> p kc f", p=128))
            w_sb[name] = t

        # --- persistent projection outputs ---
        qT = sb.tile([128, NJ, S], F32R, name="qT", tag="qT")     # [f%128, f//128, s]
        kT = sb.tile([128, NJ, S], F32R, name="kT", tag="kT")
        vaug = sb.tile([128, HPC, NT, D + 1], F32R, name="vaug", tag="vaug")
        for _h in range(HPC):
            for _t in range(NT):
                nc.vector.memset(vaug[:, _h, _t, D:D + 1].bitcast(F32), 1.0)

        # --- q^T / k^T projections ---
        for name, x, dst in (("wq", xq, qT), ("wk", xk, kT)):
            for j in range(NJ):
                xt = xs.tile([128, KC, 512], F32R, name=f"x_{name}_{j}", tag="xt")
                nc.sync.dma_start(
                    out=xt,
                    in_=x[:, j * 512:(j + 1) * 512].rearrange(
                        "(kc p) f -> p kc f", p=128),
                )
                for m in range(4):  # output-feature chunks of 128
                    acc = ps.tile([128, 512], F32, name=f"p_{name}_{j}_{m}",
                                  tag="proj")
                    for kc in range(KC):
                        nc.tensor.matmul(
                            acc,
                            lhsT=w_sb[name][:, kc, m * 128:(m + 1) * 128],
                            rhs=xt[:, kc, :],
                            start=(kc == 0), stop=(kc == KC - 1),
                        )
                    nc.vector.tensor_copy(
                        out=dst[:, m, j * 512:(j + 1) * 512], in_=acc)

        # --- v projection (natural [s, f] layout) into v_aug ---
        for j in range(NJ):
            xt = xs.tile([128, KC, 512], F32R, name=f"x_v_{j}", tag="xt")
            nc.sync.dma_start(
                out=xt,
                in_=xv[:, j * 512:(j + 1) * 512].rearrange(
                    "(kc p) f -> p kc f", p=128),
            )
            for sc in range(4):  # s chunks of 128 inside this j
                t = j * 4 + sc
                acc = ps.tile([128, FPC], F32, name=f"p_v_{j}_{sc}", tag="proj")
                for kc in range(KC):
                    nc.tensor.matmul(
                        acc,
                        lhsT=xt[:, kc, sc * 128:(sc + 1) * 128],
                        rhs=w_sb["wv"][:, kc, :],
                        start=(kc == 0), stop=(kc == KC - 1),
                    )
                for h in range(HPC):
                    nc.vector.tensor_copy(
                        out=vaug[:, h, t, 0:D], in_=acc[:, h * D:(h + 1) * D])

        # --- attention ---
        for h in range(HPC):
            po = (h % 2) * 64   # partition offset of head h inside its chunk
            m = h // 2
            for j in range(NJ):
                cacc = ps.tile([D + 1, 512], F32, name=f"ctx_{h}_{j}", tag="ctx")
                for tg in range(NT // 2):   # exp over 2 banks at a time
                    st = ps.tile([128, 2, 512], F32, name=f"st_{h}_{j}_{tg}",
                                 tag="st")
                    for u in range(2):
                        t = tg * 2 + u
                        nc.tensor.matmul(
                            st[:, u, :],
                            lhsT=kT[po:po + 64, m, t * 128:(t + 1) * 128],
                            rhs=qT[po:po + 64, m, j * 512:(j + 1) * 512],
                            start=True, stop=True,
                        )
                    ex = exp.tile([128, 2, 512], F32R, name=f"ex_{h}_{j}_{tg}",
                                  tag="ex")
                    nc.scalar.activation(ex, st, EXP, scale=SCALE)
                    for u in range(2):
                        t = tg * 2 + u
                        nc.tensor.matmul(
                            cacc,
                            lhsT=vaug[:, h, t, :],
                            rhs=ex[:, u, :],
                            start=(t == 0), stop=(t == NT - 1),
                        )
                csb = exp.tile([D + 1, 512], F32, name=f"csb_{h}_{j}",
                               tag="csb")
                nc.vector.tensor_copy(out=csb, in_=cacc)
                nc.sync.dma_start(
                    out=out[h, :, j * 512:(j + 1) * 512], in_=csb)

    nc.compile()
    nc.freeze()
    return nc


_NC_CACHE = None


def _get_nc():
    global _NC_CACHE
    if _NC_CACHE is None:
        _NC_CACHE = build_bass()
    return _NC_CACHE


def make_in_maps(queries, keys, values, Wq, Wk, Wv):
    # Host-side shard prep: transpose activations once per batch, slice W by head.
    xq_t = [np.ascontiguousarray(queries[b].T) for b in range(B)]
    xk_t = [np.ascontiguousarray(keys[b].T) for b in range(B)]
    xv_t = [np.ascontiguousarray(values[b].T) for b in range(B)]
    w_half = [
        (np.ascontiguousarray(Wq[:, g * FPC:(g + 1) * FPC]),
         np.ascontiguousarray(Wk[:, g * FPC:(g + 1) * FPC]),
         np.ascontiguousarray(Wv[:, g * FPC:(g + 1) * FPC]))
        for g in range(2)
    ]

    in_maps = []
    for c in range(N_CORES):
        b, g = c // 2, c % 2
        in_maps.append({
            "xq_t": xq_t[b], "xk_t": xk_t[b], "xv_t": xv_t[b],
            "wq": w_half[g][0], "wk": w_half[g][1], "wv": w_half[g][2],
        })
    return in_maps


def kernel(queries, keys, values, Wq, Wk, Wv, **_):
    queries = np.asarray(queries, dtype=np.float32)
    keys = np.asarray(keys, dtype=np.float32)
    values = np.asarray(values, dtype=np.float32)
    Wq = np.asarray(Wq, dtype=np.float32)
    Wk = np.asarray(Wk, dtype=np.float32)
    Wv = np.asarray(Wv, dtype=np.float32)

    in_maps = make_in_maps(queries, keys, values, Wq, Wk, Wv)
    nc = _get_nc()
    res = run_bass_kernel_spmd(nc, in_maps, list(range(N_CORES)))

    full = np.empty((B, S, H * D), dtype=np.float32)
    for c in range(N_CORES):
        b, g = c // 2, c % 2
        o = res.results[c]["out"]          # [HPC, D+1, S]
        ctx = o[:, :D, :] / o[:, D:D + 1, :]     # [HPC, D, S]
        dst = full[b].reshape(S, H, D)
        dst[:, g * HPC:(g + 1) * HPC, :] = ctx.transpose(2, 0, 1)
    return full



# revision 7
# speedup vs baseline: 1.7050x; 1.7050x over previous
"""Multi-head attention (B=4, S=2048, E=1024, H=16, D=64) on 8 TRN2 NeuronCores.

Sharding: core c handles batch b = c//2 and heads [8*(c%2), 8*(c%2)+8) —
data parallel over batch, tensor parallel over heads. No collectives:
each core computes its own output slice, gathered on host.

v2: all-bf16 dataflow (inputs cast on host; f32 PSUM accumulation),
N=1024 moving operands, per-head attention pipeline with a 2-deep
scores-PSUM ring (4 banks) + one [65, 2048] context accumulator
(4 banks). Softmax denominator via an all-ones 65th column of the
value matrix; division done on host.

Per-core algorithm:
  qT[f, s] / kT[f, s] = W^T X^T  (feature-major, head pairs split the
  128-partition dim: head 2m on partitions 0-63, 2m+1 on 64-127)
  vaug[sk, h, t, 0:64] = V; vaug[.., 64] = 1
  per head h, sk-block t (128 wide), sq-half u (1024 wide):
    st[sk, sq]  = kT_h[:, t]^T @ qT_h[:, u]          (K=64, N=1024)
    ex          = exp(st / 8)                         (ACT, bf16 out)
    ctx[0:65, u] += vaug_h[t]^T @ ex                  (K=128, N=1024)
  out[h] = ctx  ([65, 2048] bf16; row 64 = softmax denominator)
"""

import numpy as np
import ml_dtypes
from contextlib import ExitStack

import concourse.bass as bass
import concourse.tile as tile
from concourse import bacc
from concourse import mybir
from concourse.bass_utils import run_bass_kernel_spmd

F32 = mybir.dt.float32
BF16 = mybir.dt.bfloat16
EXP = mybir.ActivationFunctionType.Exp
NP_BF16 = ml_dtypes.bfloat16

B, S, E = 4, 2048, 1024
H, D = 16, 64
HPC = 8            # heads per core
FPC = HPC * D      # 512 output features per core
N_CORES = 8
KC = E // 128      # contraction chunks
NT = S // 128      # sk blocks
SCALE = 0.125      # 1/sqrt(64)


def build_bass():
    nc = bacc.Bacc()
    xq = nc.declare_dram_parameter("xq_t", [E, S], BF16, isOutput=False)
    xk = nc.declare_dram_parameter("xk_t", [E, S], BF16, isOutput=False)
    xv = nc.declare_dram_parameter("xv_t", [E, S], BF16, isOutput=False)
    wq = nc.declare_dram_parameter("wq", [E, FPC], BF16, isOutput=False)
    wk = nc.declare_dram_parameter("wk", [E, FPC], BF16, isOutput=False)
    wv = nc.declare_dram_parameter("wv", [E, FPC], BF16, isOutput=False)
    out = nc.declare_dram_parameter("out", [HPC, D + 1, S], BF16, isOutput=True)

    with tile.TileContext(nc) as tc, ExitStack() as ctx:
        sb = ctx.enter_context(tc.tile_pool(name="sb", bufs=1))
        exp = ctx.enter_context(tc.tile_pool(name="exp", bufs=3))
        csb = ctx.enter_context(tc.tile_pool(name="csb", bufs=2))
        # --- weights + activations, resident in SBUF (bf16) ---
        w_sb = {}
        for name, w in (("wq", wq), ("wk", wk), ("wv", wv)):
            t = sb.tile([128, KC, FPC], BF16, name=f"{name}_sb", tag=f"{name}_sb")
            nc.sync.dma_start(out=t, in_=w.rearrange("(kc p) f -> p kc f", p=128))
            w_sb[name] = t
        x_sb = {}
        for name, x in (("xk", xk), ("xq", xq), ("xv", xv)):
            t = sb.tile([128, KC, S], BF16, name=f"{name}_sb", tag=f"{name}_sb")
            nc.sync.dma_start(out=t, in_=x.rearrange("(kc p) s -> p kc s", p=128))
            x_sb[name] = t

        # --- persistent projection outputs ---
        qT = sb.tile([128, 4, S], BF16, name="qT", tag="qT")   # [f%128, f//128, s]
        kT = sb.tile([128, 4, S], BF16, name="kT", tag="kT")
        vaug = sb.tile([128, HPC, NT, D + 1], BF16, name="vaug", tag="vaug")
        nc.vector.memset(vaug[:, :, :, D:D + 1], 1.0)

        with tc.tile_pool(name="pproj", bufs=4, space="PSUM") as pproj:
            # --- k^T / q^T projections (f32 PSUM, bf16 out) ---
            for name, dst in (("k", kT), ("q", qT)):
                for m in range(4):
                    for u in range(4):
                        acc = pproj.tile([128, 512], F32,
                                         name=f"p_{name}_{m}_{u}", tag="proj")
                        for kc in range(KC):
                            nc.tensor.matmul(
                                acc,
                                lhsT=w_sb[f"w{name}"][:, kc, m * 128:(m + 1) * 128],
                                rhs=x_sb[f"x{name}"][:, kc, u * 512:(u + 1) * 512],
                                start=(kc == 0), stop=(kc == KC - 1),
                            )
                        nc.vector.tensor_copy(
                            out=dst[:, m, u * 512:(u + 1) * 512], in_=acc)

            # --- v projection (natural [s, f] layout) into v_aug ---
            for sc in range(NT):
                acc = pproj.tile([128, FPC], F32, name=f"p_v_{sc}", tag="projv")
                for kc in range(KC):
                    nc.tensor.matmul(
                        acc,
                        lhsT=x_sb["xv"][:, kc, sc * 128:(sc + 1) * 128],
                        rhs=w_sb["wv"][:, kc, :],
                        start=(kc == 0), stop=(kc == KC - 1),
                    )
                nc.vector.tensor_copy(
                    out=vaug[:, :, sc, 0:D],
                    in_=acc.rearrange("p (h d) -> p h d", h=HPC))

        # --- attention ---
        stp = ctx.enter_context(tc.tile_pool(name="stp", bufs=2, space="PSUM"))
        ctxp = ctx.enter_context(tc.tile_pool(name="ctxp", bufs=1, space="PSUM"))
        for h in range(HPC):
            po = (h % 2) * 64   # partition offset of head h inside its chunk
            m = h // 2
            cacc = ctxp.tile([D + 1, S], F32, name=f"ctx_{h}", tag="ctx")
            for t in range(NT):
                for u in range(2):
                    st = stp.tile([128, 2, 512], F32, name=f"st_{h}_{t}_{u}",
                                  tag="st")
                    for v in range(2):
                        j = 2 * u + v
                        nc.tensor.matmul(
                            st[:, v, :],
                            lhsT=kT[po:po + 64, m, t * 128:(t + 1) * 128],
                            rhs=qT[po:po + 64, m, j * 512:(j + 1) * 512],
                            start=True, stop=True,
                        )
                    ex = exp.tile([128, 2, 512], BF16, name=f"ex_{h}_{t}_{u}",
                                  tag="ex")
                    nc.scalar.activation(ex, st, EXP, scale=SCALE)
                    for v in range(2):
                        j = 2 * u + v
                        nc.tensor.matmul(
                            cacc[:, j * 512:(j + 1) * 512],
                            lhsT=vaug[:, h, t, :],
                            rhs=ex[:, v, :],
                            start=(t == 0), stop=(t == NT - 1),
                        )
            cs = csb.tile([D + 1, S], BF16, name=f"csb_{h}", tag="csb")
            nc.vector.tensor_copy(out=cs, in_=cacc)
            nc.sync.dma_start(out=out[h, :, :], in_=cs)

    nc.compile()
    nc.freeze()
    return nc


_NC_CACHE = None


def _get_nc():
    global _NC_CACHE
    if _NC_CACHE is None:
        _NC_CACHE = build_bass()
    return _NC_CACHE


def make_in_maps(queries, keys, values, Wq, Wk, Wv):
    # Host-side shard prep: transpose + cast to bf16 once per batch,
    # slice W by head group.
    xq_t = [np.ascontiguousarray(queries[b].T).astype(NP_BF16) for b in range(B)]
    xk_t = [np.ascontiguousarray(keys[b].T).astype(NP_BF16) for b in range(B)]
    xv_t = [np.ascontiguousarray(values[b].T).astype(NP_BF16) for b in range(B)]
    w_half = [
        (np.ascontiguousarray(Wq[:, g * FPC:(g + 1) * FPC]).astype(NP_BF16),
         np.ascontiguousarray(Wk[:, g * FPC:(g + 1) * FPC]).astype(NP_BF16),
         np.ascontiguousarray(Wv[:, g * FPC:(g + 1) * FPC]).astype(NP_BF16))
        for g in range(2)
    ]

    in_maps = []
    for c in range(N_CORES):
        b, g = c // 2, c % 2
        in_maps.append({
            "xq_t": xq_t[b], "xk_t": xk_t[b], "xv_t": xv_t[b],
            "wq": w_half[g][0], "wk": w_half[g][1], "wv": w_half[g][2],
        })
    return in_maps


def kernel(queries, keys, values, Wq, Wk, Wv, **_):
    queries = np.asarray(queries, dtype=np.float32)
    keys = np.asarray(keys, dtype=np.float32)
    values = np.asarray(values, dtype=np.float32)
    Wq = np.asarray(Wq, dtype=np.float32)
    Wk = np.asarray(Wk, dtype=np.float32)
    Wv = np.asarray(Wv, dtype=np.float32)

    in_maps = make_in_maps(queries, keys, values, Wq, Wk, Wv)
    nc = _get_nc()
    res = run_bass_kernel_spmd(nc, in_maps, list(range(N_CORES)))

    full = np.empty((B, S, H * D), dtype=np.float32)
    for c in range(N_CORES):
        b, g = c // 2, c % 2
        o = res.results[c]["out"].astype(np.float32)   # [HPC, D+1, S]
        ctx = o[:, :D, :] / o[:, D:D + 1, :]           # [HPC, D, S]
        dst = full[b].reshape(S, H, D)
        dst[:, g * HPC:(g + 1) * HPC, :] = ctx.transpose(2, 0, 1)
    return full


# revision 10
# speedup vs baseline: 2.8099x; 1.6480x over previous
"""Multi-head attention (B=4, S=2048, E=1024, H=16, D=64) on 8 TRN2 NeuronCores.

Sharding: core c handles batch b = c//2 and heads [8*(c%2), 8*(c%2)+8) —
data parallel over batch, tensor parallel over heads. No collectives:
each core computes its own output slice, gathered on host.

v3: all-bf16 dataflow (inputs cast on host; f32 PSUM accumulation).
Projections are split into 8-matmul groups; a prelude computes what the
first attention head needs, the rest are interleaved into the attention
loop where the Activation engine (exp) is the bottleneck and the PE has
slack. PSUM budget: scores ring 2x[128,2,512] (4 banks) + context
accumulator [65,1024] (2 banks) + projection accumulator ring (2 banks).

Per-core algorithm:
  qT[f, s] / kT[f, s] = W^T X^T  (feature-major; head 2m on partitions
  0-63 of chunk m, head 2m+1 on 64-127)
  vaug[sk, h, t, 0:64] = V; vaug[.., 64] = 1
  per head h, sq-half u, sk-block t:
    st[sk, j]   = kT_h[:, t]^T @ qT_h[:, j],  j = 2u, 2u+1   (K=64, N=512)
    ex          = exp(st / 8)                 (one ACT per t, bf16 out)
    ctx[0:65, j] += vaug_h[t]^T @ ex[:, j]    (K=128, N=512)
  out[h] = ctx  ([65, 2048] bf16; row 64 = softmax denominator, divided
  out on host)
"""

import numpy as np
import ml_dtypes
from contextlib import ExitStack

import concourse.bass as bass
import concourse.tile as tile
from concourse import bacc
from concourse import mybir
from concourse.bass_utils import run_bass_kernel_spmd

F32 = mybir.dt.float32
BF16 = mybir.dt.bfloat16
EXP = mybir.ActivationFunctionType.Exp
NP_BF16 = ml_dtypes.bfloat16

B, S, E = 4, 2048, 1024
H, D = 16, 64
HPC = 8            # heads per core
FPC = HPC * D      # 512 output features per core
N_CORES = 8
KC = E // 128      # contraction chunks
NT = S // 128      # sk blocks
SCALE = 0.125      # 1/sqrt(64)


def build_bass():
    nc = bacc.Bacc()
    xq = nc.declare_dram_parameter("xq_t", [E, S], BF16, isOutput=False)
    xk = nc.declare_dram_parameter("xk_t", [E, S], BF16, isOutput=False)
    xv = nc.declare_dram_parameter("xv_t", [E, S], BF16, isOutput=False)
    wq = nc.declare_dram_parameter("wq", [E, FPC], BF16, isOutput=False)
    wk = nc.declare_dram_parameter("wk", [E, FPC], BF16, isOutput=False)
    wv = nc.declare_dram_parameter("wv", [E, FPC], BF16, isOutput=False)
    out = nc.declare_dram_parameter("out", [HPC, D + 1, S], BF16, isOutput=True)

    with tile.TileContext(nc) as tc, ExitStack() as ctx:
        sb = ctx.enter_context(tc.tile_pool(name="sb", bufs=1))
        exp = ctx.enter_context(tc.tile_pool(name="exp", bufs=3))
        csb = ctx.enter_context(tc.tile_pool(name="csb", bufs=2))
        pproj = ctx.enter_context(tc.tile_pool(name="pproj", bufs=2,
                                               space="PSUM"))

        # --- weights + activations, resident in SBUF (bf16) ---
        w_sb = {}
        for name, w in (("wq", wq), ("wk", wk), ("wv", wv)):
            t = sb.tile([128, KC, FPC], BF16, name=f"{name}_sb", tag=f"{name}_sb")
            nc.sync.dma_start(out=t, in_=w.rearrange("(kc p) f -> p kc f", p=128))
            w_sb[name] = t
        x_sb = {}
        for name, x in (("xk", xk), ("xq", xq), ("xv", xv)):
            t = sb.tile([128, KC, S], BF16, name=f"{name}_sb", tag=f"{name}_sb")
            nc.sync.dma_start(out=t, in_=x.rearrange("(kc p) s -> p kc s", p=128))
            x_sb[name] = t

        # --- persistent projection outputs ---
        qT = sb.tile([128, 4, S], BF16, name="qT", tag="qT")   # [f%128, f//128, s]
        kT = sb.tile([128, 4, S], BF16, name="kT", tag="kT")
        vaug = sb.tile([128, HPC, NT, D + 1], BF16, name="vaug", tag="vaug")
        nc.vector.memset(vaug[:, :, :, D:D + 1], 1.0)

        # --- projection group emitters (8 accumulating matmuls + 1 copy) ---
        def qk_group(name, dst, m, u):
            def emit():
                acc = pproj.tile([128, 512], F32, name=f"p_{name}_{m}_{u}",
                                 tag="proj")
                for kc in range(KC):
                    nc.tensor.matmul(
                        acc,
                        lhsT=w_sb[f"w{name}"][:, kc, m * 128:(m + 1) * 128],
                        rhs=x_sb[f"x{name}"][:, kc, u * 512:(u + 1) * 512],
                        start=(kc == 0), stop=(kc == KC - 1),
                    )
                nc.vector.tensor_copy(
                    out=dst[:, m, u * 512:(u + 1) * 512], in_=acc)
            return emit

        def v_group(sc):
            def emit():
                acc = pproj.tile([128, FPC], F32, name=f"p_v_{sc}", tag="proj")
                for kc in range(KC):
                    nc.tensor.matmul(
                        acc,
                        lhsT=x_sb["xv"][:, kc, sc * 128:(sc + 1) * 128],
                        rhs=w_sb["wv"][:, kc, :],
                        start=(kc == 0), stop=(kc == KC - 1),
                    )
                nc.vector.tensor_copy(
                    out=vaug[:, :, sc, 0:D],
                    in_=acc.rearrange("p (h d) -> p h d", h=HPC))
            return emit

        # Prelude: everything head 0 needs up front.
        for uu in range(4):
            qk_group("k", kT, 0, uu)()
        for uu in range(4):
            qk_group("q", qT, 0, uu)()
        for sc in range(8):
            v_group(sc)()

        # Deferred: remaining v blocks first (consumed by head 0's t loop),
        # then k/q chunks m=1..3 (first consumed by head 2m).
        deferred = [v_group(sc) for sc in range(8, NT)]
        for mm in range(1, 4):
            for name, dst in (("k", kT), ("q", qT)):
                for uu in range(4):
                    deferred.append(qk_group(name, dst, mm, uu))

        # --- attention: head pairs (2m, 2m+1) share each sk-block's scores
        # matmuls as two concurrent K=64 row-tiles of the PE array ---
        stp = ctx.enter_context(tc.tile_pool(name="stp", bufs=2, space="PSUM"))
        ctxp = ctx.enter_context(tc.tile_pool(name="ctxp", bufs=1, space="PSUM"))
        it = 0   # global t-iteration index, for deferred-group scheduling
        for m in range(4):
            for j in range(4):   # sq quarter (512 wide)
                ca = ctxp.tile([D + 1, 512], F32, name=f"ctxa_{m}_{j}",
                               tag="ctxa")
                cb = ctxp.tile([D + 1, 512], F32, name=f"ctxb_{m}_{j}",
                               tag="ctxb")
                for t in range(NT):
                    st = stp.tile([128, 2, 512], F32, name=f"st_{m}_{j}_{t}",
                                  tag="st")
                    for v in range(2):
                        po = v * 64
                        nc.tensor.matmul(
                            st[:, v, :],
                            lhsT=kT[po:po + 64, m, t * 128:(t + 1) * 128],
                            rhs=qT[po:po + 64, m, j * 512:(j + 1) * 512],
                            start=True, stop=True,
                        )
                    ex = exp.tile([128, 2, 512], BF16, name=f"ex_{m}_{j}_{t}",
                                  tag="ex")
                    nc.scalar.activation(ex, st, EXP, scale=SCALE)
                    for v, cc in ((0, ca), (1, cb)):
                        nc.tensor.matmul(
                            cc,
                            lhsT=vaug[:, 2 * m + v, t, :],
                            rhs=ex[:, v, :],
                            start=(t == 0), stop=(t == NT - 1),
                        )
                    # interleave deferred projection work into PE slack
                    if deferred and (it < 16 or it % 5 == 1):
                        deferred.pop(0)()
                    it += 1
                for v, cc in ((0, ca), (1, cb)):
                    cs = csb.tile([D + 1, 512], BF16, name=f"csb_{m}_{j}_{v}",
                                  tag="csb")
                    nc.vector.tensor_copy(out=cs, in_=cc)
                    nc.sync.dma_start(
                        out=out[2 * m + v, :, j * 512:(j + 1) * 512], in_=cs)
        while deferred:
            deferred.pop(0)()

    nc.compile()
    nc.freeze()
    return nc


_NC_CACHE = None


def _get_nc():
    global _NC_CACHE
    if _NC_CACHE is None:
        _NC_CACHE = build_bass()
    return _NC_CACHE


def make_in_maps(queries, keys, values, Wq, Wk, Wv):
    # Host-side shard prep: transpose + cast to bf16 once per batch,
    # slice W by head group.
    xq_t = [np.ascontiguousarray(queries[b].T).astype(NP_BF16) for b in range(B)]
    xk_t = [np.ascontiguousarray(keys[b].T).astype(NP_BF16) for b in range(B)]
    xv_t = [np.ascontiguousarray(values[b].T).astype(NP_BF16) for b in range(B)]
    w_half = [
        (np.ascontiguousarray(Wq[:, g * FPC:(g + 1) * FPC]).astype(NP_BF16),
         np.ascontiguousarray(Wk[:, g * FPC:(g + 1) * FPC]).astype(NP_BF16),
         np.ascontiguousarray(Wv[:, g * FPC:(g + 1) * FPC]).astype(NP_BF16))
        for g in range(2)
    ]

    in_maps = []
    for c in range(N_CORES):
        b, g = c // 2, c % 2
        in_maps.append({
            "xq_t": xq_t[b], "xk_t": xk_t[b], "xv_t": xv_t[b],
            "wq": w_half[g][0], "wk": w_half[g][1], "wv": w_half[g][2],
        })
    return in_maps


def kernel(queries, keys, values, Wq, Wk, Wv, **_):
    queries = np.asarray(queries, dtype=np.float32)
    keys = np.asarray(keys, dtype=np.float32)
    values = np.asarray(values, dtype=np.float32)
    Wq = np.asarray(Wq, dtype=np.float32)
    Wk = np.asarray(Wk, dtype=np.float32)
    Wv = np.asarray(Wv, dtype=np.float32)

    in_maps = make_in_maps(queries, keys, values, Wq, Wk, Wv)
    nc = _get_nc()
    res = run_bass_kernel_spmd(nc, in_maps, list(range(N_CORES)))

    full = np.empty((B, S, H * D), dtype=np.float32)
    for c in range(N_CORES):
        b, g = c // 2, c % 2
        o = res.results[c]["out"].astype(np.float32)   # [HPC, D+1, S]
        ctx = o[:, :D, :] / o[:, D:D + 1, :]           # [HPC, D, S]
        dst = full[b].reshape(S, H, D)
        dst[:, g * HPC:(g + 1) * HPC, :] = ctx.transpose(2, 0, 1)
    return full
